# revision 23
# baseline (speedup 1.0000x reference)
"""Multi-head attention block (B=4, L=S=2048, D=P=1024, H=8) on 8 TRN2 cores.

Sharding: core c = 2*b + g handles batch b and head-group g (4 heads).
  - Wq/Wk/Wv column slice [1024, 512], Wo row slice [512, 1024].
  - Each core computes a partial output [2048, 1024] (its head-group's
    contribution through Wo) in bf16; the host sums the two partials per batch
    and adds bo (the tensor-parallel all-reduce done at unshard time).

The host pre-transposes everything into the exact SBUF layouts (and casts to
bf16), so the device does only plain contiguous DMA loads — no xbar-transpose
DMAs on the input path (v1 spent ~112us of serialized DMA_TRANSPOSE there).

Per-core kernel:
  1. xq/xk/xv arrive chunk-major feature-transposed ([128, c k t] with
     x[p, 4096c+512k+t] = X[512c+t, 128k+p]); 1MB chunk loads straight into
     the X^T chunk tensors. Weights arrive in w_sb layout ([128, (k o)]).
  2. q^T, k^T [512, 2048] feature-major (lhsT=W, rhs=X^T) + bias;
     v [2048, 512] token-major (lhsT=X^T, rhs=W) + bias, stored interleaved
     with a ones-column per head (v_aug [2048, 4*132]) so the attention
     row-sums fall out of the ctx matmul for free.
  3. Per (l-half, head): scores^T tiles [s=128, l=1024] on PE; exp on ACT
     (scale=1/sqrt(128)) -> E^T bf16; ctx[l, 129] accumulated over s in
     PSUM accumulators packed 3-per-bank (start=True on the first matmul of
     each bank clears the whole bank's has_written, so the packed neighbors'
     first start=False writes overwrite cleanly — no zeroing matmuls);
     col 128 is the softmax denominator; normalize with DVE reciprocal+scale.
  4. ctx_norm [2048, 512] bf16 -> xbar-transpose -> ctx^T; out-projection
     (lhsT=ctx^T, rhs=Wo) -> partial out bf16 [2048, 1024] -> DRAM.
"""

import sys

sys.path.insert(0, "/opt/trn_rl_repo")

import math

import numpy as np

import concourse.bass as bass
import concourse.tile as tile
from concourse import bacc, mybir
from concourse.bass_utils import run_bass_kernel_spmd

F32 = mybir.dt.float32
BF16 = mybir.dt.bfloat16

TOK = 2048          # tokens per core (one batch), 16 tiles of 128
DF = 1024           # model dim, 8 k-tiles of 128
PF = 512            # per-core projection width (4 heads x 128)
NHEAD = 4           # heads per core
EH = 128            # head dim
VSTRIDE = 132       # v_aug per-head stride (128 v cols + 1 ones col + 3 pad)
SCALE = 1.0 / math.sqrt(128.0)

T16 = TOK // 128    # 16 token tiles
K8 = DF // 128      # 8 feature k-tiles
M4 = PF // 128      # 4 outf tiles == heads
N4 = TOK // 512     # 4 token chunks of 512
LHALF = 2           # two l-halves of 1024
CHW = K8 * 512      # chunk width in the chunk-major x layout


def _build():
    nc = bacc.Bacc("TRN2", target_bir_lowering=False, debug=False, num_devices=8)

    # x*: chunk-major X^T: x[p, 4096c + 512k + t] = X[512c + t, 128k + p]
    xq = nc.dram_tensor("xq", [128, N4 * CHW], BF16, kind="ExternalInput")
    xk = nc.dram_tensor("xk", [128, N4 * CHW], BF16, kind="ExternalInput")
    xv = nc.dram_tensor("xv", [128, N4 * CHW], BF16, kind="ExternalInput")
    # w*: w[p, 512k + o] = W[128k + p, o] (o = out-feature within the slice)
    wq = nc.dram_tensor("wq", [128, K8 * PF], BF16, kind="ExternalInput")
    wk = nc.dram_tensor("wk", [128, K8 * PF], BF16, kind="ExternalInput")
    wv = nc.dram_tensor("wv", [128, K8 * PF], BF16, kind="ExternalInput")
    # wo: wo[p, 1024f + o] = Wo_slice[128f + p, o] (f = ctx feature tile)
    wo = nc.dram_tensor("wo", [128, M4 * DF], BF16, kind="ExternalInput")
    bq = nc.dram_tensor("bq", [PF], F32, kind="ExternalInput")
    bk = nc.dram_tensor("bk", [PF], F32, kind="ExternalInput")
    bv = nc.dram_tensor("bv", [PF], F32, kind="ExternalInput")
    out = nc.dram_tensor("out", [TOK, DF], BF16, kind="ExternalOutput")

    with tile.TileContext(nc) as tc:
        with tc.tile_pool(name="sb", bufs=1) as sb, \
             tc.tile_pool(name="ps", bufs=1, space="PSUM") as ps:

            # ---- biases -------------------------------------------------
            # bq/bk as [128, 4] f32: column m = bias slice for outf tile m.
            bq_sb = sb.tile([128, M4], F32, tag="bq_sb")
            bk_sb = sb.tile([128, M4], F32, tag="bk_sb")
            nc.gpsimd.dma_start(bq_sb[:], bq[:].rearrange("(m p) -> p m", p=128))
            nc.gpsimd.dma_start(bk_sb[:], bk[:].rearrange("(m p) -> p m", p=128))
            # bv broadcast to all 128 partitions via K=1 fp32 matmul.
            bv_row = sb.tile([1, PF], F32, tag="bv_row")
            nc.gpsimd.dma_start(bv_row[:], bv[:].rearrange("(o n) -> o n", o=1))
            ones1 = sb.tile([1, 128], F32, tag="ones1")
            nc.vector.memset(ones1[:], 1.0)
            bvb_ps = ps.tile([128, 512], F32, tag="out_ps", bufs=1)
            nc.tensor.matmul(bvb_ps[:], ones1[:], bv_row[:], start=True, stop=True)
            bvb = sb.tile([128, PF], F32, tag="bvb")
            nc.vector.tensor_copy(bvb[:], bvb_ps[:])

            # ---- weights (already in SBUF layout on host) ---------------
            wv_sb = sb.tile([128, K8 * PF], BF16, tag="wv_sb", name="wv_sb")
            wq_sb = sb.tile([128, K8 * PF], BF16, tag="wq_sb", name="wq_sb")
            wk_sb = sb.tile([128, K8 * PF], BF16, tag="wk_sb", name="wk_sb")
            wo_sb = sb.tile([128, M4 * DF], BF16, tag="wo_sb", name="wo_sb")

            # ---- persistent activation tensors -------------------------
            qT = [sb.tile([128, TOK], BF16, tag=f"qT{m}", name=f"qT{m}") for m in range(M4)]
            kT = [sb.tile([128, TOK], BF16, tag=f"kT{m}", name=f"kT{m}") for m in range(M4)]
            v_aug = [sb.tile([128, NHEAD * VSTRIDE], BF16, tag=f"va{t}", name=f"va{t}")
                     for t in range(T16)]
            for t in range(T16):
                nc.vector.memset(v_aug[t][:], 1.0)
            # ctxT: 4 chunk tensors of 512 tokens, feature f-tile at
            # cols [512f, 512f+512) within each chunk
            ctxTc = [sb.tile([128, M4 * 512], BF16, tag=f"cT{c}", name=f"cT{c}")
                     for c in range(N4)]

            # ---- X^T chunk loads (plain contiguous DMAs) ----------------
            def load_xT(x_dram, split_first=False, first_eng=None):
                xTc = [sb.tile([128, CHW], BF16, tag="xT", bufs=8,
                               name=f"xTc{c}") for c in range(N4)]
                for c in range(N4):
                    if c == 0 and split_first:
                        # halves align with k-stripes 0-3 / 4-7, so the first
                        # projection group's k<4 matmuls start one half-DMA
                        # earlier
                        hw = CHW // 2
                        (first_eng or nc.sync).dma_start(
                            xTc[0][:, :hw], x_dram[:, :hw])
                        nc.sync.dma_start(xTc[0][:, hw:], x_dram[:, hw:CHW])
                        continue
                    nc.sync.dma_start(xTc[c][:], x_dram[:, CHW * c:CHW * (c + 1)])

                def xt_ap(k, lo, width):
                    cc, off = divmod(lo, 512)
                    assert off + width <= 512
                    return xTc[cc][:, 512 * k + off:512 * k + off + width]
                return xt_ap

            def proj_group(xT, w_sb, b_sb, dstT, m, n, ps_tag, ps_bufs):
                # one token-chunk group: dstT[m][:, 512n:512(n+1)] =
                # (X @ W + b)^T chunk, feature-major
                pst = ps.tile([128, 512], F32, tag=ps_tag, bufs=ps_bufs)
                for k in range(K8):
                    nc.tensor.matmul(
                        pst[:],
                        w_sb[:, PF * k + 128 * m:PF * k + 128 * (m + 1)],
                        xT(k, 512 * n, 512),
                        start=(k == 0), stop=(k == K8 - 1),
                    )
                nc.vector.tensor_scalar_add(
                    dstT[m][:, 512 * n:512 * (n + 1)], pst[:],
                    b_sb[:, m:m + 1],
                )

            def v_group(xT, t, ps_tag, ps_bufs):
                # one token tile of the v projection (token-major + bias),
                # interleaved into v_aug beside the ones columns
                pst = ps.tile([128, 512], F32, tag=ps_tag, bufs=ps_bufs)
                for k in range(K8):
                    nc.tensor.matmul(
                        pst[:],
                        xT(k, 128 * t, 128),
                        wv_sb[:, PF * k:PF * (k + 1)],
                        start=(k == 0), stop=(k == K8 - 1),
                    )
                for h in range(NHEAD):
                    nc.vector.tensor_add(
                        v_aug[t][:, VSTRIDE * h:VSTRIDE * h + 128],
                        pst[:, 128 * h:128 * (h + 1)],
                        bvb[:, 128 * h:128 * (h + 1)],
                    )

            # PE warm-up: ~3.4us of dummy matmuls into the out_ps bank while
            # the first DMAs are in flight, so the HAM clock-gate is already
            # at 2.4GHz (warm) when the first projection group issues.
            for _ in range(28):
                wmm = ps.tile([128, 512], F32, tag="out_ps", bufs=1)
                nc.tensor.matmul(wmm[:, 0:128], ones1[:], ones1[:],
                                 start=True, stop=True)

            # DMA order v, q, k matches the prep consumption order — the
            # emitted static schedule executes in-order per engine, so PE
            # must consume chunks in arrival order or it stalls.
            # The first weight/chunk pieces ride SWDGE (gpsimd): its queue
            # delivers first bytes ~4us before the HWDGE ring spins up.
            nc.gpsimd.dma_start(wv_sb[:, :K8 * PF // 2], wv[:, :K8 * PF // 2])
            nc.sync.dma_start(wv_sb[:, K8 * PF // 2:], wv[:, K8 * PF // 2:])
            xvT = load_xT(xv, split_first=True, first_eng=nc.gpsimd)
            nc.sync.dma_start(wq_sb[:], wq[:])
            xqT = load_xT(xq)
            nc.sync.dma_start(wk_sb[:], wk[:])
            xkT = load_xT(xk)
            nc.sync.dma_start(wo_sb[:], wo[:])
            for t in range(T16):
                v_group(xvT, t, "att_ps", 2)
            del xvT
            for n in range(N4):
                proj_group(xqT, wq_sb, bq_sb, qT, 0, n, "att_ps", 2)
            for n in range(N4):
                proj_group(xkT, wk_sb, bk_sb, kT, 0, n, "att_ps", 2)

            # ---- attention + out-projection ----------------------------
            # Filler work (remaining v groups, next heads' q/k projections,
            # lh0 out-projection pieces) is laced INTO the attention s-loops
            # at a fixed cadence. The Tile scheduler emits a static
            # per-engine order from a cost-model simulation; coarse
            # emission blocks give it freedom to mis-order around real
            # hardware timing, which turns into in-order stalls. Explicit
            # fine-grained lacing pins a known-good order, and keeps each
            # filler's DVE drain ahead of the head's normalize ops in the
            # strict-FIFO DVE queue.
            ctxn = [None] * T16

            def attention_head(lh, h, fillers=()):
                # 8 ctx accumulators [128, 129] packed 3 per PSUM bank.
                # The first matmul into each bank uses start=True: it clears
                # has_written for the WHOLE bank, so the other accumulators'
                # first (start=False) writes overwrite-on-clear correctly.
                cps = [ps.tile([128, 512], F32, tag="ctx_ps", bufs=3,
                               name=f"cps{lh}_{h}_{_}") for _ in range(3)]

                def acc(j):
                    i, jj = divmod(j, 3)
                    return cps[i][:, 129 * jj:129 * jj + 129]

                popped = 0
                for s in range(T16):
                    sc = ps.tile([128, 1024], F32, tag="att_ps", bufs=2)
                    for c2 in range(2):
                        nc.tensor.matmul(
                            sc[:, 512 * c2:512 * (c2 + 1)],
                            kT[h][:, 128 * s:128 * (s + 1)],
                            qT[h][:, 1024 * lh + 512 * c2:
                                     1024 * lh + 512 * (c2 + 1)],
                            start=True, stop=True,
                        )
                    eT = sb.tile([128, 1024], BF16, tag="eT", bufs=8)
                    nc.scalar.activation(
                        eT[:], sc[:], mybir.ActivationFunctionType.Exp,
                        scale=SCALE,
                    )
                    for j in range(8):
                        nc.tensor.matmul(
                            acc(j),
                            eT[:, 128 * j:128 * (j + 1)],
                            v_aug[s][:, VSTRIDE * h:VSTRIDE * h + 129],
                            start=(s == 0 and j % 3 == 0), stop=(s == T16 - 1),
                            skip_group_check=True,
                        )
                    # evenly spread filler emission across the s-loop
                    target = len(fillers) * (s + 1) // T16
                    while popped < target:
                        fillers[popped]()
                        popped += 1
                for j in range(8):
                    t = 8 * lh + j
                    if ctxn[t] is None:
                        ctxn[t] = sb.tile([128, PF], BF16, tag="ctxn",
                                          bufs=17, name=f"ctxn{t}")
                    rs = sb.tile([128, 1], F32, tag="rs", bufs=4)
                    nc.vector.reciprocal(rs[:], acc(j)[:, 128:129])
                    nc.vector.tensor_scalar_mul(
                        ctxn[t][:, 128 * h:128 * (h + 1)],
                        acc(j)[:, 0:128], rs[:, 0:1],
                    )
                # transpose this head's ctx blocks right away (per-head
                # [128,128] xbar transposes): they overlap the next head's
                # attention, so at the last head only its own 8 small
                # transposes gate outproj. They stay off the ACT queue while
                # exps remain (a waiting DMA trigger would block the strict
                # FIFO ahead of the next head's exps); only the final head
                # splits across both HWDGE rings.
                last = lh == 1 and h == NHEAD - 1
                for j in range(8):
                    t = 8 * lh + j
                    cc, ttt = divmod(t, 4)
                    eng = nc.scalar if (last and j % 2 == 1) else nc.sync
                    eng.dma_start(
                        ctxTc[cc][:, 512 * h + 128 * ttt:
                                   512 * h + 128 * (ttt + 1)],
                        ctxn[t][:, 128 * h:128 * (h + 1)],
                        transpose=True,
                    )

            def outproj_pso(lh, j, n2, ps_tag="out_ps", ps_bufs=1):
                t = 8 * lh + j
                pso = ps.tile([128, 512], F32, tag=ps_tag, bufs=ps_bufs)
                for kf in range(M4):
                    nc.tensor.matmul(
                        pso[:],
                        ctxTc[t // 4][:, 512 * kf + 128 * (t % 4):
                                      512 * kf + 128 * (t % 4) + 128],
                        wo_sb[:, DF * kf + 512 * n2:DF * kf + 512 * (n2 + 1)],
                        start=(kf == 0), stop=(kf == M4 - 1),
                    )
                osb = sb.tile([128, 512], BF16, tag="osb", bufs=4)
                if lh == 1 and (2 * j + n2) % 2 == 0:
                    nc.scalar.copy(osb[:], pso[:])
                else:
                    nc.vector.tensor_copy(osb[:], pso[:])
                # the tail (lh1) stores ride HWDGE (lower latency) so the
                # last store isn't behind SWDGE's ~2us setup
                eng = nc.scalar if lh == 1 else nc.gpsimd
                eng.dma_start(
                    out[128 * t:128 * (t + 1), 512 * n2:512 * (n2 + 1)],
                    osb[:],
                )

            def F(fn, *a):
                return lambda: fn(*a)

            # lh0: att(0,h) laced with head h+1's q/k projection groups.
            # All fillers chain through the 1-bank out_ps ring; attention's
            # sc ring is untouched.
            for h in range(NHEAD - 1):
                attention_head(0, h, fillers=(
                    [F(proj_group, xqT, wq_sb, bq_sb, qT, h + 1, n, "out_ps", 1)
                     for n in range(N4)]
                    + [F(proj_group, xkT, wk_sb, bk_sb, kT, h + 1, n, "out_ps", 1)
                       for n in range(N4)]))
            attention_head(0, NHEAD - 1)
            del xqT, xkT
            # lh1: att(1,0..2) laced with the lh0 out-projection pieces (their
            # PSUM->SBUF casts then sit ahead of each head's normalize in the
            # strict-FIFO DVE queue instead of head-of-line-blocking it).
            attention_head(1, 0, fillers=[
                F(outproj_pso, 0, j, n2) for j in range(0, 3) for n2 in range(2)])
            attention_head(1, 1, fillers=[
                F(outproj_pso, 0, j, n2) for j in range(3, 6) for n2 in range(2)])
            attention_head(1, 2, fillers=[
                F(outproj_pso, 0, j, n2) for j in range(6, 8) for n2 in range(2)])
            attention_head(1, 3)
            for j in range(8):
                for n2 in range(2):
                    outproj_pso(1, j, n2, ps_tag="ctx_ps", ps_bufs=3)

    nc.finalize()
    return nc


_NC_CACHE = None


def _get_nc():
    global _NC_CACHE
    if _NC_CACHE is None:
        _NC_CACHE = _build()
    return _NC_CACHE


def _make_in_maps(queries, keys, values, Wq, bq, Wk, bk, Wv, bv, Wo):
    import ml_dtypes

    BF = ml_dtypes.bfloat16

    def c(a):
        return np.ascontiguousarray(a)

    def xT_chunks(X):
        # [2048, 1024] f32 -> [128, 16384] bf16, x[p, 4096c+512k+t] = X[512c+t, 128k+p]
        a = np.asarray(X, np.float32).astype(BF)
        a = a.reshape(N4, 512, K8, 128).transpose(3, 0, 2, 1)
        return c(a.reshape(128, N4 * CHW))

    def w_lay(W):
        # [1024, 512] -> [128, 4096] bf16, w[p, 512k + o] = W[128k+p, o]
        a = np.asarray(W, np.float32).astype(BF)
        a = a.reshape(K8, 128, PF).transpose(1, 0, 2)
        return c(a.reshape(128, K8 * PF))

    def wo_lay(W):
        # [512, 1024] -> [128, 4096] bf16, wo[p, 1024f + o] = W[128f+p, o]
        a = np.asarray(W, np.float32).astype(BF)
        a = a.reshape(M4, 128, DF).transpose(1, 0, 2)
        return c(a.reshape(128, M4 * DF))

    # X layouts are shared by the two cores of each batch — build once.
    xqs = [xT_chunks(queries[b]) for b in range(4)]
    xks = [xT_chunks(keys[b]) for b in range(4)]
    xvs = [xT_chunks(values[b]) for b in range(4)]
    in_maps = []
    for core in range(8):
        b, g = divmod(core, 2)
        sl = slice(512 * g, 512 * (g + 1))
        in_maps.append({
            "xq": xqs[b],
            "xk": xks[b],
            "xv": xvs[b],
            "wq": w_lay(Wq[:, sl]), "wk": w_lay(Wk[:, sl]), "wv": w_lay(Wv[:, sl]),
            "wo": wo_lay(Wo[sl, :]),
            "bq": c(bq[sl]), "bk": c(bk[sl]), "bv": c(bv[sl]),
        })
    return in_maps


def _run(trace=False, **inputs):
    arrs = {k: np.asarray(v, dtype=np.float32) for k, v in inputs.items()}
    nc = _get_nc()
    in_maps = _make_in_maps(
        arrs["queries"], arrs["keys"], arrs["values"],
        arrs["Wq"], arrs["bq"], arrs["Wk"], arrs["bk"],
        arrs["Wv"], arrs["bv"], arrs["Wo"],
    )
    res = run_bass_kernel_spmd(nc, in_maps, core_ids=list(range(8)), trace=trace)
    bo = arrs["bo"]
    full = np.empty((4, TOK, DF), np.float32)
    for b in range(4):
        full[b] = (res.results[2 * b]["out"].astype(np.float32)
                   + res.results[2 * b + 1]["out"].astype(np.float32) + bo)
    return full, res


def kernel(**inputs) -> np.ndarray:
    full, _ = _run(trace=False, **inputs)
    return full


# revision 24
# speedup vs baseline: 1.0282x; 1.0282x over previous
"""Multi-head attention block (B=4, L=S=2048, D=P=1024, H=8) on 8 TRN2 cores.

Sharding: core c = 2*b + g handles batch b and head-group g (4 heads).
  - Wq/Wk/Wv column slice [1024, 512], Wo row slice [512, 1024].
  - Each core computes a partial output [2048, 1024] (its head-group's
    contribution through Wo) in bf16; the host sums the two partials per batch
    and adds bo (the tensor-parallel all-reduce done at unshard time).

The host pre-transposes everything into the exact SBUF layouts (and casts to
bf16), so the device does only plain contiguous DMA loads — no xbar-transpose
DMAs on the input path (v1 spent ~112us of serialized DMA_TRANSPOSE there).

Per-core kernel:
  1. xq/xk/xv arrive chunk-major feature-transposed ([128, c k t] with
     x[p, 4096c+512k+t] = X[512c+t, 128k+p]); 1MB chunk loads straight into
     the X^T chunk tensors. Weights arrive in w_sb layout ([128, (k o)]).
  2. q^T, k^T [512, 2048] feature-major (lhsT=W, rhs=X^T) + bias;
     v [2048, 512] token-major (lhsT=X^T, rhs=W) + bias, stored interleaved
     with a ones-column per head (v_aug [2048, 4*132]) so the attention
     row-sums fall out of the ctx matmul for free.
  3. Per (l-half, head): scores^T tiles [s=128, l=1024] on PE; exp on ACT
     (scale=1/sqrt(128)) -> E^T bf16; ctx[l, 129] accumulated over s in
     PSUM accumulators packed 3-per-bank (start=True on the first matmul of
     each bank clears the whole bank's has_written, so the packed neighbors'
     first start=False writes overwrite cleanly — no zeroing matmuls);
     col 128 is the softmax denominator; normalize with DVE reciprocal+scale.
  4. ctx_norm [2048, 512] bf16 -> xbar-transpose -> ctx^T; out-projection
     (lhsT=ctx^T, rhs=Wo) -> partial out bf16 [2048, 1024] -> DRAM.
"""

import sys

sys.path.insert(0, "/opt/trn_rl_repo")

import math

import numpy as np

import concourse.bass as bass
import concourse.tile as tile
from concourse import bacc, mybir
from concourse.bass_utils import run_bass_kernel_spmd

F32 = mybir.dt.float32
BF16 = mybir.dt.bfloat16

TOK = 2048          # tokens per core (one batch), 16 tiles of 128
DF = 1024           # model dim, 8 k-tiles of 128
PF = 512            # per-core projection width (4 heads x 128)
NHEAD = 4           # heads per core
EH = 128            # head dim
VSTRIDE = 132       # v_aug per-head stride (128 v cols + 1 ones col + 3 pad)
SCALE = 1.0 / math.sqrt(128.0)

T16 = TOK // 128    # 16 token tiles
K8 = DF // 128      # 8 feature k-tiles
M4 = PF // 128      # 4 outf tiles == heads
N4 = TOK // 512     # 4 token chunks of 512
LHALF = 2           # two l-halves of 1024
CHW = K8 * 512      # chunk width in the chunk-major x layout


def _build():
    nc = bacc.Bacc("TRN2", target_bir_lowering=False, debug=False, num_devices=8)

    # x*: chunk-major X^T: x[p, 4096c + 512k + t] = X[512c + t, 128k + p]
    xq = nc.dram_tensor("xq", [128, N4 * CHW], BF16, kind="ExternalInput")
    xk = nc.dram_tensor("xk", [128, N4 * CHW], BF16, kind="ExternalInput")
    xv = nc.dram_tensor("xv", [128, N4 * CHW], BF16, kind="ExternalInput")
    # w*: w[p, 512k + o] = W[128k + p, o] (o = out-feature within the slice)
    wq = nc.dram_tensor("wq", [128, K8 * PF], BF16, kind="ExternalInput")
    wk = nc.dram_tensor("wk", [128, K8 * PF], BF16, kind="ExternalInput")
    wv = nc.dram_tensor("wv", [128, K8 * PF], BF16, kind="ExternalInput")
    # wo: wo[p, 1024f + o] = Wo_slice[128f + p, o] (f = ctx feature tile)
    wo = nc.dram_tensor("wo", [128, M4 * DF], BF16, kind="ExternalInput")
    bq = nc.dram_tensor("bq", [PF], F32, kind="ExternalInput")
    bk = nc.dram_tensor("bk", [PF], F32, kind="ExternalInput")
    bv = nc.dram_tensor("bv", [PF], F32, kind="ExternalInput")
    out = nc.dram_tensor("out", [TOK, DF], BF16, kind="ExternalOutput")

    with tile.TileContext(nc) as tc:
        with tc.tile_pool(name="sb", bufs=1) as sb, \
             tc.tile_pool(name="ps", bufs=1, space="PSUM") as ps:

            # ---- biases -------------------------------------------------
            # bq/bk as [128, 4] f32: column m = bias slice for outf tile m.
            bq_sb = sb.tile([128, M4], F32, tag="bq_sb")
            bk_sb = sb.tile([128, M4], F32, tag="bk_sb")
            nc.gpsimd.dma_start(bq_sb[:], bq[:].rearrange("(m p) -> p m", p=128))
            nc.gpsimd.dma_start(bk_sb[:], bk[:].rearrange("(m p) -> p m", p=128))
            # bv broadcast to all 128 partitions via K=1 fp32 matmul.
            bv_row = sb.tile([1, PF], F32, tag="bv_row")
            nc.gpsimd.dma_start(bv_row[:], bv[:].rearrange("(o n) -> o n", o=1))
            ones1 = sb.tile([1, 128], F32, tag="ones1")
            nc.vector.memset(ones1[:], 1.0)
            bvb_ps = ps.tile([128, 512], F32, tag="out_ps", bufs=1)
            nc.tensor.matmul(bvb_ps[:], ones1[:], bv_row[:], start=True, stop=True)
            bvb = sb.tile([128, PF], F32, tag="bvb")
            nc.vector.tensor_copy(bvb[:], bvb_ps[:])

            # ---- weights (already in SBUF layout on host) ---------------
            wv_sb = sb.tile([128, K8 * PF], BF16, tag="wv_sb", name="wv_sb")
            wq_sb = sb.tile([128, K8 * PF], BF16, tag="wq_sb", name="wq_sb")
            wk_sb = sb.tile([128, K8 * PF], BF16, tag="wk_sb", name="wk_sb")
            wo_sb = sb.tile([128, M4 * DF], BF16, tag="wo_sb", name="wo_sb")

            # ---- persistent activation tensors -------------------------
            qT = [sb.tile([128, TOK], BF16, tag=f"qT{m}", name=f"qT{m}") for m in range(M4)]
            kT = [sb.tile([128, TOK], BF16, tag=f"kT{m}", name=f"kT{m}") for m in range(M4)]
            v_aug = [sb.tile([128, NHEAD * VSTRIDE], BF16, tag=f"va{t}", name=f"va{t}")
                     for t in range(T16)]
            for t in range(T16):
                nc.vector.memset(v_aug[t][:], 1.0)
            # ctxT: 4 chunk tensors of 512 tokens, feature f-tile at
            # cols [512f, 512f+512) within each chunk
            ctxTc = [sb.tile([128, M4 * 512], BF16, tag=f"cT{c}", name=f"cT{c}")
                     for c in range(N4)]

            # ---- X^T chunk loads (plain contiguous DMAs) ----------------
            def load_xT(x_dram, split_first=False, first_eng=None):
                xTc = [sb.tile([128, CHW], BF16, tag="xT", bufs=8,
                               name=f"xTc{c}") for c in range(N4)]
                for c in range(N4):
                    if c == 0 and split_first:
                        # halves align with k-stripes 0-3 / 4-7, so the first
                        # projection group's k<4 matmuls start one half-DMA
                        # earlier
                        hw = CHW // 2
                        (first_eng or nc.sync).dma_start(
                            xTc[0][:, :hw], x_dram[:, :hw])
                        nc.sync.dma_start(xTc[0][:, hw:], x_dram[:, hw:CHW])
                        continue
                    nc.sync.dma_start(xTc[c][:], x_dram[:, CHW * c:CHW * (c + 1)])

                def xt_ap(k, lo, width):
                    cc, off = divmod(lo, 512)
                    assert off + width <= 512
                    return xTc[cc][:, 512 * k + off:512 * k + off + width]
                return xt_ap

            def proj_group(xT, w_sb, b_sb, dstT, m, n, ps_tag, ps_bufs):
                # one token-chunk group: dstT[m][:, 512n:512(n+1)] =
                # (X @ W + b)^T chunk, feature-major
                pst = ps.tile([128, 512], F32, tag=ps_tag, bufs=ps_bufs)
                for k in range(K8):
                    nc.tensor.matmul(
                        pst[:],
                        w_sb[:, PF * k + 128 * m:PF * k + 128 * (m + 1)],
                        xT(k, 512 * n, 512),
                        start=(k == 0), stop=(k == K8 - 1),
                    )
                nc.vector.tensor_scalar_add(
                    dstT[m][:, 512 * n:512 * (n + 1)], pst[:],
                    b_sb[:, m:m + 1],
                )

            def v_group(xT, t, ps_tag, ps_bufs):
                # one token tile of the v projection (token-major + bias),
                # interleaved into v_aug beside the ones columns
                pst = ps.tile([128, 512], F32, tag=ps_tag, bufs=ps_bufs)
                for k in range(K8):
                    nc.tensor.matmul(
                        pst[:],
                        xT(k, 128 * t, 128),
                        wv_sb[:, PF * k:PF * (k + 1)],
                        start=(k == 0), stop=(k == K8 - 1),
                    )
                for h in range(NHEAD):
                    nc.vector.tensor_add(
                        v_aug[t][:, VSTRIDE * h:VSTRIDE * h + 128],
                        pst[:, 128 * h:128 * (h + 1)],
                        bvb[:, 128 * h:128 * (h + 1)],
                    )

            # PE warm-up: ~3.4us of dummy matmuls into the out_ps bank while
            # the first DMAs are in flight, so the HAM clock-gate is already
            # at 2.4GHz (warm) when the first projection group issues.
            for _ in range(28):
                wmm = ps.tile([128, 512], F32, tag="out_ps", bufs=1)
                nc.tensor.matmul(wmm[:, 0:128], ones1[:], ones1[:],
                                 start=True, stop=True)

            # DMA order v, q, k matches the prep consumption order — the
            # emitted static schedule executes in-order per engine, so PE
            # must consume chunks in arrival order or it stalls.
            nc.sync.dma_start(wv_sb[:, :K8 * PF // 2], wv[:, :K8 * PF // 2])
            nc.sync.dma_start(wv_sb[:, K8 * PF // 2:], wv[:, K8 * PF // 2:])
            xvT = load_xT(xv, split_first=True)
            nc.sync.dma_start(wq_sb[:], wq[:])
            xqT = load_xT(xq)
            nc.sync.dma_start(wk_sb[:], wk[:])
            xkT = load_xT(xk)
            nc.sync.dma_start(wo_sb[:], wo[:])
            for t in range(T16):
                v_group(xvT, t, "att_ps", 2)
            del xvT
            for n in range(N4):
                proj_group(xqT, wq_sb, bq_sb, qT, 0, n, "att_ps", 2)
            for n in range(N4):
                proj_group(xkT, wk_sb, bk_sb, kT, 0, n, "att_ps", 2)

            # ---- attention + out-projection ----------------------------
            # Filler work (remaining v groups, next heads' q/k projections,
            # lh0 out-projection pieces) is laced INTO the attention s-loops
            # at a fixed cadence. The Tile scheduler emits a static
            # per-engine order from a cost-model simulation; coarse
            # emission blocks give it freedom to mis-order around real
            # hardware timing, which turns into in-order stalls. Explicit
            # fine-grained lacing pins a known-good order, and keeps each
            # filler's DVE drain ahead of the head's normalize ops in the
            # strict-FIFO DVE queue.
            ctxn = [None] * T16

            def attention_head(lh, h, fillers=()):
                # 8 ctx accumulators [128, 129] packed 3 per PSUM bank.
                # The first matmul into each bank uses start=True: it clears
                # has_written for the WHOLE bank, so the other accumulators'
                # first (start=False) writes overwrite-on-clear correctly.
                cps = [ps.tile([128, 512], F32, tag="ctx_ps", bufs=3,
                               name=f"cps{lh}_{h}_{_}") for _ in range(3)]

                def acc(j):
                    i, jj = divmod(j, 3)
                    return cps[i][:, 129 * jj:129 * jj + 129]

                popped = 0
                for s in range(T16):
                    sc = ps.tile([128, 1024], F32, tag="att_ps", bufs=2)
                    for c2 in range(2):
                        nc.tensor.matmul(
                            sc[:, 512 * c2:512 * (c2 + 1)],
                            kT[h][:, 128 * s:128 * (s + 1)],
                            qT[h][:, 1024 * lh + 512 * c2:
                                     1024 * lh + 512 * (c2 + 1)],
                            start=True, stop=True,
                        )
                    eT = sb.tile([128, 1024], BF16, tag="eT", bufs=8)
                    nc.scalar.activation(
                        eT[:], sc[:], mybir.ActivationFunctionType.Exp,
                        scale=SCALE,
                    )
                    for j in range(8):
                        nc.tensor.matmul(
                            acc(j),
                            eT[:, 128 * j:128 * (j + 1)],
                            v_aug[s][:, VSTRIDE * h:VSTRIDE * h + 129],
                            start=(s == 0 and j % 3 == 0), stop=(s == T16 - 1),
                            skip_group_check=True,
                        )
                    # evenly spread filler emission across the s-loop
                    target = len(fillers) * (s + 1) // T16
                    while popped < target:
                        fillers[popped]()
                        popped += 1
                for j in range(8):
                    t = 8 * lh + j
                    if ctxn[t] is None:
                        ctxn[t] = sb.tile([128, PF], BF16, tag="ctxn",
                                          bufs=17, name=f"ctxn{t}")
                    rs = sb.tile([128, 1], F32, tag="rs", bufs=4)
                    nc.vector.reciprocal(rs[:], acc(j)[:, 128:129])
                    nc.vector.tensor_scalar_mul(
                        ctxn[t][:, 128 * h:128 * (h + 1)],
                        acc(j)[:, 0:128], rs[:, 0:1],
                    )
                # transpose this head's ctx blocks right away (per-head
                # [128,128] xbar transposes): they overlap the next head's
                # attention, so at the last head only its own 8 small
                # transposes gate outproj. They stay off the ACT queue while
                # exps remain (a waiting DMA trigger would block the strict
                # FIFO ahead of the next head's exps); only the final head
                # splits across both HWDGE rings.
                last = lh == 1 and h == NHEAD - 1
                for j in range(8):
                    t = 8 * lh + j
                    cc, ttt = divmod(t, 4)
                    eng = nc.scalar if (last and j % 2 == 1) else nc.sync
                    eng.dma_start(
                        ctxTc[cc][:, 512 * h + 128 * ttt:
                                   512 * h + 128 * (ttt + 1)],
                        ctxn[t][:, 128 * h:128 * (h + 1)],
                        transpose=True,
                    )

            def outproj_pso(lh, j, n2, ps_tag="out_ps", ps_bufs=1):
                t = 8 * lh + j
                pso = ps.tile([128, 512], F32, tag=ps_tag, bufs=ps_bufs)
                for kf in range(M4):
                    nc.tensor.matmul(
                        pso[:],
                        ctxTc[t // 4][:, 512 * kf + 128 * (t % 4):
                                      512 * kf + 128 * (t % 4) + 128],
                        wo_sb[:, DF * kf + 512 * n2:DF * kf + 512 * (n2 + 1)],
                        start=(kf == 0), stop=(kf == M4 - 1),
                    )
                osb = sb.tile([128, 512], BF16, tag="osb", bufs=4)
                if lh == 1 and (2 * j + n2) % 2 == 0:
                    nc.scalar.copy(osb[:], pso[:])
                else:
                    nc.vector.tensor_copy(osb[:], pso[:])
                # the tail (lh1) stores ride HWDGE (lower latency) so the
                # last store isn't behind SWDGE's ~2us setup
                eng = nc.scalar if lh == 1 else nc.gpsimd
                eng.dma_start(
                    out[128 * t:128 * (t + 1), 512 * n2:512 * (n2 + 1)],
                    osb[:],
                )

            def F(fn, *a):
                return lambda: fn(*a)

            # lh0: att(0,h) laced with head h+1's q/k projection groups.
            # All fillers chain through the 1-bank out_ps ring; attention's
            # sc ring is untouched.
            for h in range(NHEAD - 1):
                attention_head(0, h, fillers=(
                    [F(proj_group, xqT, wq_sb, bq_sb, qT, h + 1, n, "out_ps", 1)
                     for n in range(N4)]
                    + [F(proj_group, xkT, wk_sb, bk_sb, kT, h + 1, n, "out_ps", 1)
                       for n in range(N4)]))
            attention_head(0, NHEAD - 1)
            del xqT, xkT
            # lh1: att(1,0..2) laced with the lh0 out-projection pieces (their
            # PSUM->SBUF casts then sit ahead of each head's normalize in the
            # strict-FIFO DVE queue instead of head-of-line-blocking it).
            attention_head(1, 0, fillers=[
                F(outproj_pso, 0, j, n2) for j in range(0, 3) for n2 in range(2)])
            attention_head(1, 1, fillers=[
                F(outproj_pso, 0, j, n2) for j in range(3, 6) for n2 in range(2)])
            attention_head(1, 2, fillers=[
                F(outproj_pso, 0, j, n2) for j in range(6, 8) for n2 in range(2)])
            attention_head(1, 3)
            for j in range(8):
                for n2 in range(2):
                    outproj_pso(1, j, n2, ps_tag="ctx_ps", ps_bufs=3)

    nc.finalize()
    return nc


_NC_CACHE = None


def _get_nc():
    global _NC_CACHE
    if _NC_CACHE is None:
        _NC_CACHE = _build()
    return _NC_CACHE


def _make_in_maps(queries, keys, values, Wq, bq, Wk, bk, Wv, bv, Wo):
    import ml_dtypes

    BF = ml_dtypes.bfloat16

    def c(a):
        return np.ascontiguousarray(a)

    def xT_chunks(X):
        # [2048, 1024] f32 -> [128, 16384] bf16, x[p, 4096c+512k+t] = X[512c+t, 128k+p]
        a = np.asarray(X, np.float32).astype(BF)
        a = a.reshape(N4, 512, K8, 128).transpose(3, 0, 2, 1)
        return c(a.reshape(128, N4 * CHW))

    def w_lay(W):
        # [1024, 512] -> [128, 4096] bf16, w[p, 512k + o] = W[128k+p, o]
        a = np.asarray(W, np.float32).astype(BF)
        a = a.reshape(K8, 128, PF).transpose(1, 0, 2)
        return c(a.reshape(128, K8 * PF))

    def wo_lay(W):
        # [512, 1024] -> [128, 4096] bf16, wo[p, 1024f + o] = W[128f+p, o]
        a = np.asarray(W, np.float32).astype(BF)
        a = a.reshape(M4, 128, DF).transpose(1, 0, 2)
        return c(a.reshape(128, M4 * DF))

    # X layouts are shared by the two cores of each batch — build once.
    xqs = [xT_chunks(queries[b]) for b in range(4)]
    xks = [xT_chunks(keys[b]) for b in range(4)]
    xvs = [xT_chunks(values[b]) for b in range(4)]
    in_maps = []
    for core in range(8):
        b, g = divmod(core, 2)
        sl = slice(512 * g, 512 * (g + 1))
        in_maps.append({
            "xq": xqs[b],
            "xk": xks[b],
            "xv": xvs[b],
            "wq": w_lay(Wq[:, sl]), "wk": w_lay(Wk[:, sl]), "wv": w_lay(Wv[:, sl]),
            "wo": wo_lay(Wo[sl, :]),
            "bq": c(bq[sl]), "bk": c(bk[sl]), "bv": c(bv[sl]),
        })
    return in_maps


def _run(trace=False, **inputs):
    arrs = {k: np.asarray(v, dtype=np.float32) for k, v in inputs.items()}
    nc = _get_nc()
    in_maps = _make_in_maps(
        arrs["queries"], arrs["keys"], arrs["values"],
        arrs["Wq"], arrs["bq"], arrs["Wk"], arrs["bk"],
        arrs["Wv"], arrs["bv"], arrs["Wo"],
    )
    res = run_bass_kernel_spmd(nc, in_maps, core_ids=list(range(8)), trace=trace)
    bo = arrs["bo"]
    full = np.empty((4, TOK, DF), np.float32)
    for b in range(4):
        full[b] = (res.results[2 * b]["out"].astype(np.float32)
                   + res.results[2 * b + 1]["out"].astype(np.float32) + bo)
    return full, res


def kernel(**inputs) -> np.ndarray:
    full, _ = _run(trace=False, **inputs)
    return full


# revision 26
# speedup vs baseline: 1.1248x; 1.0940x over previous
"""Multi-head attention block (B=4, L=S=2048, D=P=1024, H=8) on 8 TRN2 cores.

Sharding: core c = 2*b + g handles batch b and head-group g (4 heads).
  - Wq/Wk/Wv column slice [1024, 512], Wo row slice [512, 1024].
  - Each core computes a partial output [2048, 1024] (its head-group's
    contribution through Wo) in bf16; the host sums the two partials per batch
    and adds bo (the tensor-parallel all-reduce done at unshard time).

The host pre-transposes everything into the exact SBUF layouts (and casts to
bf16), so the device does only plain contiguous DMA loads — no xbar-transpose
DMAs on the input path (v1 spent ~112us of serialized DMA_TRANSPOSE there).

Per-core kernel:
  1. xq/xk/xv arrive chunk-major feature-transposed ([128, c k t] with
     x[p, 4096c+512k+t] = X[512c+t, 128k+p]); 1MB chunk loads straight into
     the X^T chunk tensors. Weights arrive in w_sb layout ([128, (k o)]).
  2. q^T, k^T [512, 2048] feature-major (lhsT=W, rhs=X^T) + bias;
     v [2048, 512] token-major (lhsT=X^T, rhs=W) + bias, stored interleaved
     with a ones-column per head (v_aug [2048, 4*132]) so the attention
     row-sums fall out of the ctx matmul for free.
  3. Per (l-half, head): scores^T tiles [s=128, l=1024] on PE; exp on ACT
     (scale=1/sqrt(128)) -> E^T bf16; ctx[l, 129] accumulated over s in
     PSUM accumulators packed 3-per-bank (start=True on the first matmul of
     each bank clears the whole bank's has_written, so the packed neighbors'
     first start=False writes overwrite cleanly — no zeroing matmuls);
     col 128 is the softmax denominator; normalize with DVE reciprocal+scale,
     then per-head [128,128] xbar transposes into ctx^T chunk tensors.
  4. Out-projection (lhsT=ctx^T, rhs=Wo) -> partial out bf16 [2048,1024] -> DRAM.

Scheduling: the Tile scheduler emits one STATIC order per engine queue
(strict FIFO at runtime), so filler work — head h+1's q/k projection groups
during lh0 attention, lh0 out-projection pieces during lh1 attention — is
laced into the attention s-loops at a fixed cadence, keeping PE ~saturated
while the ACT exp stream (the secondary bottleneck) self-paces. Fillers
chain through the 1-bank out_ps ring so the scores (att_ps) ring is
untouched, and their DVE drains land ahead of each head's normalize ops in
the DVE FIFO (a drain stuck behind a not-yet-ready normalize would stall
the whole queue).
"""

import sys

sys.path.insert(0, "/opt/trn_rl_repo")

import math

import numpy as np

import concourse.bass as bass
import concourse.tile as tile
from concourse import bacc, mybir
from concourse.bass_utils import run_bass_kernel_spmd

F32 = mybir.dt.float32
BF16 = mybir.dt.bfloat16

TOK = 2048          # tokens per core (one batch), 16 tiles of 128
DF = 1024           # model dim, 8 k-tiles of 128
PF = 512            # per-core projection width (4 heads x 128)
NHEAD = 4           # heads per core
EH = 128            # head dim
VSTRIDE = 132       # v_aug per-head stride (128 v cols + 1 ones col + 3 pad)
SCALE = 1.0 / math.sqrt(128.0)

T16 = TOK // 128    # 16 token tiles
K8 = DF // 128      # 8 feature k-tiles
M4 = PF // 128      # 4 outf tiles == heads
N4 = TOK // 512     # 4 token chunks of 512
LHALF = 2           # two l-halves of 1024
CHW = K8 * 512      # chunk width in the chunk-major x layout


def _build():
    nc = bacc.Bacc("TRN2", target_bir_lowering=False, debug=False, num_devices=8)

    # x*: chunk-major X^T: x[p, 4096c + 512k + t] = X[512c + t, 128k + p]
    xq = nc.dram_tensor("xq", [128, N4 * CHW], BF16, kind="ExternalInput")
    xk = nc.dram_tensor("xk", [128, N4 * CHW], BF16, kind="ExternalInput")
    xv = nc.dram_tensor("xv", [128, N4 * CHW], BF16, kind="ExternalInput")
    # w*: w[p, 512k + o] = W[128k + p, o] (o = out-feature within the slice)
    wq = nc.dram_tensor("wq", [128, K8 * PF], BF16, kind="ExternalInput")
    wk = nc.dram_tensor("wk", [128, K8 * PF], BF16, kind="ExternalInput")
    wv = nc.dram_tensor("wv", [128, K8 * PF], BF16, kind="ExternalInput")
    # wo: wo[p, 1024f + o] = Wo_slice[128f + p, o] (f = ctx feature tile)
    wo = nc.dram_tensor("wo", [128, M4 * DF], BF16, kind="ExternalInput")
    bq = nc.dram_tensor("bq", [PF], F32, kind="ExternalInput")
    bk = nc.dram_tensor("bk", [PF], F32, kind="ExternalInput")
    bv = nc.dram_tensor("bv", [PF], F32, kind="ExternalInput")
    out = nc.dram_tensor("out", [TOK, DF], BF16, kind="ExternalOutput")

    with tile.TileContext(nc) as tc:
        with tc.tile_pool(name="sb", bufs=1) as sb, \
             tc.tile_pool(name="ps", bufs=1, space="PSUM") as ps:

            # ---- biases -------------------------------------------------
            # bq/bk as [128, 4] f32: column m = bias slice for outf tile m.
            bq_sb = sb.tile([128, M4], F32, tag="bq_sb")
            bk_sb = sb.tile([128, M4], F32, tag="bk_sb")
            nc.gpsimd.dma_start(bq_sb[:], bq[:].rearrange("(m p) -> p m", p=128))
            nc.gpsimd.dma_start(bk_sb[:], bk[:].rearrange("(m p) -> p m", p=128))
            # bv broadcast to all 128 partitions via K=1 fp32 matmul.
            bv_row = sb.tile([1, PF], F32, tag="bv_row")
            nc.gpsimd.dma_start(bv_row[:], bv[:].rearrange("(o n) -> o n", o=1))
            ones1 = sb.tile([1, 128], F32, tag="ones1")
            nc.vector.memset(ones1[:], 1.0)
            bvb_ps = ps.tile([128, 512], F32, tag="out_ps", bufs=1)
            nc.tensor.matmul(bvb_ps[:], ones1[:], bv_row[:], start=True, stop=True)
            bvb = sb.tile([128, PF], F32, tag="bvb")
            nc.vector.tensor_copy(bvb[:], bvb_ps[:])

            # ---- weights (already in SBUF layout on host) ---------------
            wv_sb = sb.tile([128, K8 * PF], BF16, tag="wv_sb", name="wv_sb")
            wq_sb = sb.tile([128, K8 * PF], BF16, tag="wq_sb", name="wq_sb")
            wk_sb = sb.tile([128, K8 * PF], BF16, tag="wk_sb", name="wk_sb")
            wo_sb = sb.tile([128, M4 * DF], BF16, tag="wo_sb", name="wo_sb")

            # ---- persistent activation tensors -------------------------
            qT = [sb.tile([128, TOK], BF16, tag=f"qT{m}", name=f"qT{m}") for m in range(M4)]
            kT = [sb.tile([128, TOK], BF16, tag=f"kT{m}", name=f"kT{m}") for m in range(M4)]
            v_aug = [sb.tile([128, NHEAD * VSTRIDE], BF16, tag=f"va{t}", name=f"va{t}")
                     for t in range(T16)]
            for t in range(T16):
                nc.vector.memset(v_aug[t][:], 1.0)
            # ctxT: 4 chunk tensors of 512 tokens, feature f-tile at
            # cols [512f, 512f+512) within each chunk
            ctxTc = [sb.tile([128, M4 * 512], BF16, tag=f"cT{c}", name=f"cT{c}")
                     for c in range(N4)]

            # ---- X^T chunk loads (plain contiguous DMAs) ----------------
            def load_xT(x_dram, split_first=False, first_eng=None):
                xTc = [sb.tile([128, CHW], BF16, tag="xT", bufs=8,
                               name=f"xTc{c}") for c in range(N4)]
                for c in range(N4):
                    if c == 0 and split_first:
                        # halves align with k-stripes 0-3 / 4-7, so the first
                        # projection group's k<4 matmuls start one half-DMA
                        # earlier
                        hw = CHW // 2
                        (first_eng or nc.sync).dma_start(
                            xTc[0][:, :hw], x_dram[:, :hw])
                        nc.sync.dma_start(xTc[0][:, hw:], x_dram[:, hw:CHW])
                        continue
                    nc.sync.dma_start(xTc[c][:], x_dram[:, CHW * c:CHW * (c + 1)])

                def xt_ap(k, lo, width):
                    cc, off = divmod(lo, 512)
                    assert off + width <= 512
                    return xTc[cc][:, 512 * k + off:512 * k + off + width]
                return xt_ap

            def proj_group(xT, w_sb, b_sb, dstT, m, n, ps_tag, ps_bufs):
                # one token-chunk group: dstT[m][:, 512n:512(n+1)] =
                # (X @ W + b)^T chunk, feature-major
                pst = ps.tile([128, 512], F32, tag=ps_tag, bufs=ps_bufs)
                for k in range(K8):
                    nc.tensor.matmul(
                        pst[:],
                        w_sb[:, PF * k + 128 * m:PF * k + 128 * (m + 1)],
                        xT(k, 512 * n, 512),
                        start=(k == 0), stop=(k == K8 - 1),
                    )
                nc.vector.tensor_scalar_add(
                    dstT[m][:, 512 * n:512 * (n + 1)], pst[:],
                    b_sb[:, m:m + 1],
                )

            def v_group(xT, t, ps_tag, ps_bufs):
                # one token tile of the v projection (token-major + bias),
                # interleaved into v_aug beside the ones columns
                pst = ps.tile([128, 512], F32, tag=ps_tag, bufs=ps_bufs)
                for k in range(K8):
                    nc.tensor.matmul(
                        pst[:],
                        xT(k, 128 * t, 128),
                        wv_sb[:, PF * k:PF * (k + 1)],
                        start=(k == 0), stop=(k == K8 - 1),
                    )
                for h in range(NHEAD):
                    nc.vector.tensor_add(
                        v_aug[t][:, VSTRIDE * h:VSTRIDE * h + 128],
                        pst[:, 128 * h:128 * (h + 1)],
                        bvb[:, 128 * h:128 * (h + 1)],
                    )

            # DMA order v, q, k matches the prep consumption order — the
            # emitted static schedule executes in-order per engine, so PE
            # must consume chunks in arrival order or it stalls.
            nc.sync.dma_start(wv_sb[:, :K8 * PF // 2], wv[:, :K8 * PF // 2])
            nc.sync.dma_start(wv_sb[:, K8 * PF // 2:], wv[:, K8 * PF // 2:])
            xvT = load_xT(xv, split_first=True)
            nc.sync.dma_start(wq_sb[:], wq[:])
            xqT = load_xT(xq)
            nc.sync.dma_start(wk_sb[:], wk[:])
            xkT = load_xT(xk)
            nc.sync.dma_start(wo_sb[:], wo[:])
            for t in range(T16):
                v_group(xvT, t, "att_ps", 2)
            del xvT
            for n in range(N4):
                proj_group(xqT, wq_sb, bq_sb, qT, 0, n, "att_ps", 2)
            for n in range(N4):
                proj_group(xkT, wk_sb, bk_sb, kT, 0, n, "att_ps", 2)

            # ---- attention + out-projection ----------------------------
            # Filler work (remaining v groups, next heads' q/k projections,
            # lh0 out-projection pieces) is laced INTO the attention s-loops
            # at a fixed cadence. The Tile scheduler emits a static
            # per-engine order from a cost-model simulation; coarse
            # emission blocks give it freedom to mis-order around real
            # hardware timing, which turns into in-order stalls. Explicit
            # fine-grained lacing pins a known-good order, and keeps each
            # filler's DVE drain ahead of the head's normalize ops in the
            # strict-FIFO DVE queue.
            ctxn = [None] * T16

            def attention_head(lh, h, fillers=()):
                # 8 ctx accumulators [128, 129] packed 3 per PSUM bank.
                # The first matmul into each bank uses start=True: it clears
                # has_written for the WHOLE bank, so the other accumulators'
                # first (start=False) writes overwrite-on-clear correctly.
                cps = [ps.tile([128, 512], F32, tag="ctx_ps", bufs=3,
                               name=f"cps{lh}_{h}_{_}") for _ in range(3)]

                def acc(j):
                    i, jj = divmod(j, 3)
                    return cps[i][:, 129 * jj:129 * jj + 129]

                popped = 0
                for s in range(T16):
                    sc = ps.tile([128, 1024], F32, tag="att_ps", bufs=2)
                    for c2 in range(2):
                        nc.tensor.matmul(
                            sc[:, 512 * c2:512 * (c2 + 1)],
                            kT[h][:, 128 * s:128 * (s + 1)],
                            qT[h][:, 1024 * lh + 512 * c2:
                                     1024 * lh + 512 * (c2 + 1)],
                            start=True, stop=True,
                        )
                    eT = sb.tile([128, 1024], BF16, tag="eT", bufs=8)
                    nc.scalar.activation(
                        eT[:], sc[:], mybir.ActivationFunctionType.Exp,
                        scale=SCALE,
                    )
                    for j in range(8):
                        nc.tensor.matmul(
                            acc(j),
                            eT[:, 128 * j:128 * (j + 1)],
                            v_aug[s][:, VSTRIDE * h:VSTRIDE * h + 129],
                            start=(s == 0 and j % 3 == 0), stop=(s == T16 - 1),
                            skip_group_check=True,
                        )
                    # evenly spread filler emission across the s-loop
                    target = len(fillers) * (s + 1) // T16
                    while popped < target:
                        fillers[popped]()
                        popped += 1
                for j in range(8):
                    t = 8 * lh + j
                    if ctxn[t] is None:
                        ctxn[t] = sb.tile([128, PF], BF16, tag="ctxn",
                                          bufs=17, name=f"ctxn{t}")
                    rs = sb.tile([128, 1], F32, tag="rs", bufs=4)
                    nc.vector.reciprocal(rs[:], acc(j)[:, 128:129])
                    nc.vector.tensor_scalar_mul(
                        ctxn[t][:, 128 * h:128 * (h + 1)],
                        acc(j)[:, 0:128], rs[:, 0:1],
                    )
                # transpose this head's ctx blocks right away (per-head
                # [128,128] xbar transposes): they overlap the next head's
                # attention, so at the last head only its own 8 small
                # transposes gate outproj. They stay off the ACT queue while
                # exps remain (a waiting DMA trigger would block the strict
                # FIFO ahead of the next head's exps); only the final head
                # splits across both HWDGE rings.
                last = lh == 1 and h == NHEAD - 1
                for j in range(8):
                    t = 8 * lh + j
                    cc, ttt = divmod(t, 4)
                    eng = nc.scalar if (last and j % 2 == 1) else nc.sync
                    eng.dma_start(
                        ctxTc[cc][:, 512 * h + 128 * ttt:
                                   512 * h + 128 * (ttt + 1)],
                        ctxn[t][:, 128 * h:128 * (h + 1)],
                        transpose=True,
                    )

            def outproj_pso(lh, j, n2, ps_tag="out_ps", ps_bufs=1):
                t = 8 * lh + j
                pso = ps.tile([128, 512], F32, tag=ps_tag, bufs=ps_bufs)
                for kf in range(M4):
                    nc.tensor.matmul(
                        pso[:],
                        ctxTc[t // 4][:, 512 * kf + 128 * (t % 4):
                                      512 * kf + 128 * (t % 4) + 128],
                        wo_sb[:, DF * kf + 512 * n2:DF * kf + 512 * (n2 + 1)],
                        start=(kf == 0), stop=(kf == M4 - 1),
                    )
                osb = sb.tile([128, 512], BF16, tag="osb", bufs=4)
                if lh == 1 and (2 * j + n2) % 2 == 0:
                    nc.scalar.copy(osb[:], pso[:])
                else:
                    nc.vector.tensor_copy(osb[:], pso[:])
                # the tail (lh1) stores ride HWDGE (lower latency) so the
                # last store isn't behind SWDGE's ~2us setup
                eng = nc.scalar if lh == 1 else nc.gpsimd
                eng.dma_start(
                    out[128 * t:128 * (t + 1), 512 * n2:512 * (n2 + 1)],
                    osb[:],
                )

            def F(fn, *a):
                return lambda: fn(*a)

            # lh0: att(0,h) laced with head h+1's q/k projection groups.
            # All fillers chain through the 1-bank out_ps ring; attention's
            # sc ring is untouched.
            for h in range(NHEAD - 1):
                attention_head(0, h, fillers=(
                    [F(proj_group, xqT, wq_sb, bq_sb, qT, h + 1, n, "out_ps", 1)
                     for n in range(N4)]
                    + [F(proj_group, xkT, wk_sb, bk_sb, kT, h + 1, n, "out_ps", 1)
                       for n in range(N4)]))
            attention_head(0, NHEAD - 1)
            del xqT, xkT
            # lh1: att(1,0..2) laced with the lh0 out-projection pieces (their
            # PSUM->SBUF casts then sit ahead of each head's normalize in the
            # strict-FIFO DVE queue instead of head-of-line-blocking it).
            attention_head(1, 0, fillers=[
                F(outproj_pso, 0, j, n2) for j in range(0, 3) for n2 in range(2)])
            attention_head(1, 1, fillers=[
                F(outproj_pso, 0, j, n2) for j in range(3, 6) for n2 in range(2)])
            attention_head(1, 2, fillers=[
                F(outproj_pso, 0, j, n2) for j in range(6, 8) for n2 in range(2)])
            attention_head(1, 3)
            for j in range(8):
                for n2 in range(2):
                    outproj_pso(1, j, n2, ps_tag="ctx_ps", ps_bufs=3)

    nc.finalize()
    return nc


_NC_CACHE = None


def _get_nc():
    global _NC_CACHE
    if _NC_CACHE is None:
        _NC_CACHE = _build()
    return _NC_CACHE


def _make_in_maps(queries, keys, values, Wq, bq, Wk, bk, Wv, bv, Wo):
    import ml_dtypes

    BF = ml_dtypes.bfloat16

    def c(a):
        return np.ascontiguousarray(a)

    def xT_chunks(X):
        # [2048, 1024] f32 -> [128, 16384] bf16, x[p, 4096c+512k+t] = X[512c+t, 128k+p]
        a = np.asarray(X, np.float32).astype(BF)
        a = a.reshape(N4, 512, K8, 128).transpose(3, 0, 2, 1)
        return c(a.reshape(128, N4 * CHW))

    def w_lay(W):
        # [1024, 512] -> [128, 4096] bf16, w[p, 512k + o] = W[128k+p, o]
        a = np.asarray(W, np.float32).astype(BF)
        a = a.reshape(K8, 128, PF).transpose(1, 0, 2)
        return c(a.reshape(128, K8 * PF))

    def wo_lay(W):
        # [512, 1024] -> [128, 4096] bf16, wo[p, 1024f + o] = W[128f+p, o]
        a = np.asarray(W, np.float32).astype(BF)
        a = a.reshape(M4, 128, DF).transpose(1, 0, 2)
        return c(a.reshape(128, M4 * DF))

    # X layouts are shared by the two cores of each batch — build once.
    xqs = [xT_chunks(queries[b]) for b in range(4)]
    xks = [xT_chunks(keys[b]) for b in range(4)]
    xvs = [xT_chunks(values[b]) for b in range(4)]
    in_maps = []
    for core in range(8):
        b, g = divmod(core, 2)
        sl = slice(512 * g, 512 * (g + 1))
        in_maps.append({
            "xq": xqs[b],
            "xk": xks[b],
            "xv": xvs[b],
            "wq": w_lay(Wq[:, sl]), "wk": w_lay(Wk[:, sl]), "wv": w_lay(Wv[:, sl]),
            "wo": wo_lay(Wo[sl, :]),
            "bq": c(bq[sl]), "bk": c(bk[sl]), "bv": c(bv[sl]),
        })
    return in_maps


def _run(trace=False, **inputs):
    arrs = {k: np.asarray(v, dtype=np.float32) for k, v in inputs.items()}
    nc = _get_nc()
    in_maps = _make_in_maps(
        arrs["queries"], arrs["keys"], arrs["values"],
        arrs["Wq"], arrs["bq"], arrs["Wk"], arrs["bk"],
        arrs["Wv"], arrs["bv"], arrs["Wo"],
    )
    res = run_bass_kernel_spmd(nc, in_maps, core_ids=list(range(8)), trace=trace)
    bo = arrs["bo"]
    full = np.empty((4, TOK, DF), np.float32)
    for b in range(4):
        full[b] = (res.results[2 * b]["out"].astype(np.float32)
                   + res.results[2 * b + 1]["out"].astype(np.float32) + bo)
    return full, res


def kernel(**inputs) -> np.ndarray:
    full, _ = _run(trace=False, **inputs)
    return full
